# revision 8
# baseline (speedup 1.0000x reference)
"""CoLT5 MoE layer (router + top-2-of-4 experts) on 8 TRN2 NeuronCores.

Sharding: data-parallel over B*L = 8192 tokens -> 1024 tokens per core.
Each core computes the router and all 4 expert MLPs densely for its token
shard; combine weights (softmax probs masked to top-2) zero out non-selected
experts, which is numerically identical to the reference's masked-input
formulation (anything times a 0.0 gate is 0.0).

Device layout is feature-major ([feature=partitions, tokens=free]) so both
weight matrices are used in natural [in, out] layout as the stationary matmul
operand, and no activation transposes are needed on device. The host
transposes each x shard once on the way in and the output once on the way
out (pure layout prep; all FLOPs run on device).

Precision: router + expert layer 1 run in float32r (full-rate fp32 matmul
mode), expert layer 2 in bf16 (weights cast by the load DMA, hidden
activations cast by the GELU's output dtype). Routing decisions (top-2
selection) are made from float32r-precision logits.
"""

import sys

for _p in ("/opt/trn_rl_repo",):
    if _p not in sys.path:
        sys.path.insert(0, _p)

import numpy as np

import concourse.bass as bass
import concourse.mybir as mybir
import concourse.tile as tile
from concourse.masks import make_identity
from concourse.tile import TileContext
from concourse.vector_clock import ScopedClock

F32 = mybir.dt.float32
F32R = mybir.dt.float32r
BF16 = mybir.dt.bfloat16

B, L, D, E = 4, 2048, 1024, 4
DH = 2 * D          # router hidden = 2048
H = 4 * D           # expert hidden = 4096
NCORES = 8
T = (B * L) // NCORES   # tokens per core = 1024
TT = 512                # token tile (fp32 moving-operand max)
NTT = T // TT           # 2
P = 128
KD = D // P             # 8   D tiles
MH_R = DH // P          # 16  router-hidden tiles
MH = H // P             # 32  expert-hidden tiles
NCH = T // P            # 8   token chunks (for routing math)


def _patched_drain_and_barrier(self, tick_clock, wait_clock):
    # Workaround: this walrus build rejects >1 sync-wait attached to the
    # Tile kernel-tail Drain ("Too many sync wait commands",
    # CoreV3GenImpl setupSyncWait). Hang the waits on nop carriers, one
    # wait each, then drain.
    nop_inst = self.nc.sync.nop(nofuse=True)
    wait_clock.add_sem_waits(nop_inst.ins, ScopedClock({None: tick_clock.global_clock}))
    si = nop_inst.ins.sync_info
    waits = list(si.on_wait) if si else []
    if len(waits) > 1:
        si.on_wait = waits[:1]
        for w in waits[1:]:
            extra = self.nc.sync.nop(nofuse=True)
            extra.ins.sync_info = mybir.SyncInfo(on_wait=[w], on_update=[])
    self.nc.sync.drain()
    self.nc.all_engine_barrier()
    popped = self.nc._tile_sem_poison_stack.pop()
    assert popped is self._sem_poison
    self.nc.clear_and_free_semaphores(list(self.sems.allocated().values()))
    self.nc.all_engine_barrier()


tile.TileContext._drain_and_barrier = _patched_drain_and_barrier

_SPLIT_ENGINES = {"PE", "DVE", "Activation", "Pool", "SP"}


def _split_multi_waits(raw):
    # This walrus build accepts at most ONE sync-wait per instruction
    # ("Too many sync wait commands"). Move excess waits onto same-engine
    # NoOp carriers inserted immediately before the owning instruction.
    import json as _json

    d = _json.loads(raw)
    ctr = [0]

    def fix_block(b):
        ins_list = b.get("instructions")
        if ins_list:
            new_list = []
            for ins in ins_list:
                si = ins.get("sync_info")
                waits = (si or {}).get("on_wait") or []
                if len(waits) > 1 and ins.get("engine") in _SPLIT_ENGINES:
                    for w in waits[:-1]:
                        ctr[0] += 1
                        nop = {
                            "engine": ins["engine"],
                            "ins": [],
                            "outs": [],
                            "name": f"I-wsplit-{ctr[0]}",
                            "opcode": "NoOp",
                            "sync_info": {"on_update": [], "on_wait": [w]},
                        }
                        if "debug" in ins:
                            nop["debug"] = ins["debug"]
                        new_list.append(nop)
                    si["on_wait"] = [waits[-1]]
                new_list.append(ins)
            b["instructions"] = new_list
        for sub in b.get("blocks") or []:
            fix_block(sub)

    for f in d["functions"]:
        for b in f["blocks"]:
            fix_block(b)
    return _json.dumps(d).encode()


_orig_to_json_bytes = bass.Bass.to_json_bytes


def _patched_to_json_bytes(self):
    return _split_multi_waits(_orig_to_json_bytes(self))


bass.Bass.to_json_bytes = _patched_to_json_bytes


def build_nc():
    nc = bass.Bass("TRN2", target_bir_lowering=False, debug=False)

    xT = nc.dram_tensor("xT", [D, T], F32, kind="ExternalInput")
    rW1 = nc.dram_tensor("rW1", [D, DH], F32, kind="ExternalInput")
    rb1 = nc.dram_tensor("rb1", [DH], F32, kind="ExternalInput")
    rW2 = nc.dram_tensor("rW2", [DH, E], F32, kind="ExternalInput")
    rb2 = nc.dram_tensor("rb2", [E], F32, kind="ExternalInput")
    We1 = nc.dram_tensor("We1", [E, D, H], F32, kind="ExternalInput")
    be1 = nc.dram_tensor("be1", [E, H], F32, kind="ExternalInput")
    We2 = nc.dram_tensor("We2", [E, H, D], F32, kind="ExternalInput")
    be2 = nc.dram_tensor("be2", [E, D], F32, kind="ExternalInput")
    outT = nc.dram_tensor("outT", [D, T], F32, kind="ExternalOutput")

    AF = mybir.ActivationFunctionType
    ALU = mybir.AluOpType
    AX = mybir.AxisListType

    with TileContext(nc) as tc:
        from contextlib import ExitStack

        ctx = ExitStack()
        with ctx:
            # ---- long-lived pools -------------------------------------
            const = ctx.enter_context(tc.tile_pool(name="const", bufs=1))
            persist = ctx.enter_context(tc.tile_pool(name="persist", bufs=1))
            w1pool = ctx.enter_context(tc.tile_pool(name="w1pool", bufs=2))
            w2pool = ctx.enter_context(tc.tile_pool(name="w2pool", bufs=2))
            rhpool = ctx.enter_context(tc.tile_pool(name="rhpool", bufs=3))
            ytpool = ctx.enter_context(tc.tile_pool(name="ytpool", bufs=3))
            pl1 = ctx.enter_context(tc.tile_pool(name="pl1", bufs=3, space="PSUM"))
            pl2 = ctx.enter_context(tc.tile_pool(name="pl2", bufs=3, space="PSUM"))

            ident = const.tile([P, P], F32, tag="ident")
            make_identity(nc, ident)

            # biases, partition-major
            rb1_sb = const.tile([P, MH_R], F32, tag="rb1")
            nc.sync.dma_start(out=rb1_sb[:], in_=rb1.ap().rearrange("(a p) -> p a", p=P))
            rb2_sb = const.tile([E, 1], F32, tag="rb2")
            nc.sync.dma_start(out=rb2_sb[:], in_=rb2.ap().rearrange("(p a) -> p a", a=1))
            be1_sb = const.tile([P, E, MH], F32, tag="be1")
            nc.sync.dma_start(out=be1_sb[:], in_=be1.ap().rearrange("e (a p) -> p e a", p=P))
            be2_sb = const.tile([P, E, KD], F32, tag="be2")
            nc.sync.dma_start(out=be2_sb[:], in_=be2.ap().rearrange("e (a p) -> p e a", p=P))

            # resident activations
            xT_sb = persist.tile([P, KD, T], F32R, tag="xT")
            nc.gpsimd.dma_start(out=xT_sb[:], in_=xT.ap().rearrange("(kd p) t -> p kd t", p=P))
            h_bf = persist.tile([P, MH, T], BF16, tag="h")
            out_acc = persist.tile([P, KD, T], F32, tag="outacc")
            w_bcast = persist.tile([P, E, T], F32, tag="wbc")

            rW2_sb = const.tile([P, MH_R, E], F32R, tag="rW2")
            nc.gpsimd.dma_start(out=rW2_sb[:], in_=rW2.ap().rearrange("(a p) e -> p a e", p=P))

            # ---- router ----------------------------------------------
            with tc.tile_pool(name="plr", bufs=2, space="PSUM") as plr:
                logits_ps = [
                    plr.tile([E, TT], F32, tag="logits", name=f"logits{tt}")
                    for tt in range(NTT)
                ]
                for mh in range(MH_R):
                    w1blk = w1pool.tile([P, KD, P], F32R, tag="w1blk")
                    nc.gpsimd.dma_start(
                        out=w1blk[:],
                        in_=rW1.ap()[:, mh * P : (mh + 1) * P].rearrange(
                            "(kd p) h -> p kd h", p=P
                        ),
                    )
                    w1r = w1blk[:]
                    for tt in range(NTT):
                        ps1 = pl1.tile([P, TT], F32, tag="ps1")
                        for kd in range(KD):
                            nc.tensor.matmul(
                                ps1[:],
                                w1r[:, kd, :],
                                xT_sb[:, kd, tt * TT : (tt + 1) * TT],
                                start=(kd == 0),
                                stop=(kd == KD - 1),
                            )
                        rh_t = rhpool.tile([P, TT], F32R, tag="rh")
                        nc.scalar.activation(
                            rh_t[:], ps1[:], AF.Gelu, bias=rb1_sb[:, mh : mh + 1]
                        )
                        nc.tensor.matmul(
                            logits_ps[tt][:],
                            rW2_sb[:, mh, :],
                            rh_t[:],
                            start=(mh == 0),
                            stop=(mh == MH_R - 1),
                            skip_group_check=True,
                        )
                logits_sb = persist.tile([E, T], F32, tag="logits_sb")
                for tt in range(NTT):
                    nc.scalar.activation(
                        logits_sb[:, tt * TT : (tt + 1) * TT],
                        logits_ps[tt][:],
                        AF.Identity,
                        bias=rb2_sb[:],
                    )

            # ---- routing math (token-major) ---------------------------
            with (
                tc.tile_pool(name="ptp", bufs=2, space="PSUM") as ptp,
                tc.tile_pool(name="route", bufs=1) as route,
            ):
                ltm = route.tile([P, NCH, E], F32, tag="ltm")
                for c in range(NCH):
                    tp = ptp.tile([P, P], F32, tag="tp")
                    nc.tensor.transpose(
                        tp[:, :E], logits_sb[:, c * P : (c + 1) * P], ident[:E, :E]
                    )
                    nc.scalar.copy(ltm[:, c, :], tp[:, :E])

                m0 = route.tile([P, NCH, 1], F32, tag="m0")
                nc.vector.reduce_max(m0[:], ltm[:], axis=AX.X)
                sh = route.tile([P, NCH, E], F32, tag="sh")
                nc.vector.tensor_sub(sh[:], ltm[:], m0[:].to_broadcast([P, NCH, E]))
                ex = route.tile([P, NCH, E], F32, tag="ex")
                nc.scalar.activation(ex[:], sh[:], AF.Exp)
                ssum = route.tile([P, NCH, 1], F32, tag="ssum")
                nc.vector.reduce_sum(ssum[:], ex[:], axis=AX.X)
                rec = route.tile([P, NCH, 1], F32, tag="rec")
                nc.vector.reciprocal(rec[:], ssum[:])
                probs = route.tile([P, NCH, E], F32, tag="probs")
                nc.vector.tensor_mul(probs[:], ex[:], rec[:].to_broadcast([P, NCH, E]))

                m1 = route.tile([P, NCH, 1], F32, tag="m1")
                nc.vector.reduce_max(m1[:], probs[:], axis=AX.X)
                selmax = route.tile([P, NCH, E], F32, tag="selmax")
                nc.vector.tensor_tensor(
                    out=selmax[:], in0=probs[:], in1=m1[:].to_broadcast([P, NCH, E]),
                    op=ALU.is_ge,
                )
                masked = route.tile([P, NCH, E], F32, tag="masked")
                nc.vector.tensor_scalar_mul(selmax[:], selmax[:], 2.0)
                nc.vector.tensor_sub(masked[:], probs[:], selmax[:])
                m2 = route.tile([P, NCH, 1], F32, tag="m2")
                nc.vector.reduce_max(m2[:], masked[:], axis=AX.X)
                sel = route.tile([P, NCH, E], F32, tag="sel")
                nc.vector.tensor_tensor(
                    out=sel[:], in0=probs[:], in1=m2[:].to_broadcast([P, NCH, E]),
                    op=ALU.is_ge,
                )
                combine = route.tile([P, NCH, E], F32, tag="combine")
                nc.vector.tensor_mul(combine[:], probs[:], sel[:])

                # transpose-and-broadcast combine back to feature-major:
                # out[m, t] = combine[t, e] via a matmul whose stationary
                # operand is one combine column broadcast along its free dim
                # (step 0), against the identity.
                for e in range(E):
                    for c in range(NCH):
                        bc = ptp.tile([P, P], F32, tag="tp")
                        nc.tensor.matmul(
                            bc[:],
                            combine[:, c, e : e + 1].to_broadcast([P, P]),
                            ident[:],
                            start=True,
                            stop=True,
                        )
                        nc.scalar.copy(w_bcast[:, e, c * P : (c + 1) * P], bc[:])

            # ---- experts ---------------------------------------------
            for e in range(E):
                for mh in range(MH):
                    w1blk = w1pool.tile([P, KD, P], F32R, tag="w1blk")
                    nc.gpsimd.dma_start(
                        out=w1blk[:],
                        in_=We1.ap()[e, :, mh * P : (mh + 1) * P].rearrange(
                            "(kd p) h -> p kd h", p=P
                        ),
                    )
                    w1r = w1blk[:]
                    for tt in range(NTT):
                        ps1 = pl1.tile([P, TT], F32, tag="ps1")
                        for kd in range(KD):
                            nc.tensor.matmul(
                                ps1[:],
                                w1r[:, kd, :],
                                xT_sb[:, kd, tt * TT : (tt + 1) * TT],
                                start=(kd == 0),
                                stop=(kd == KD - 1),
                            )
                        nc.scalar.activation(
                            h_bf[:, mh, tt * TT : (tt + 1) * TT],
                            ps1[:],
                            AF.Gelu,
                            bias=be1_sb[:, e, mh : mh + 1],
                        )
                for md in range(KD):
                    w2blk = w2pool.tile([P, MH, P], BF16, tag="w2blk")
                    nc.gpsimd.dma_start(
                        out=w2blk[:],
                        in_=We2.ap()[e, :, md * P : (md + 1) * P].rearrange(
                            "(mh p) d -> p mh d", p=P
                        ),
                    )
                    for tt in range(NTT):
                        ps2 = pl2.tile([P, TT], F32, tag="ps2")
                        for mh in range(MH):
                            nc.tensor.matmul(
                                ps2[:],
                                w2blk[:, mh, :],
                                h_bf[:, mh, tt * TT : (tt + 1) * TT],
                                start=(mh == 0),
                                stop=(mh == MH - 1),
                            )
                        y_sb = ytpool.tile([P, TT], F32, tag="ysb")
                        nc.scalar.activation(
                            y_sb[:], ps2[:], AF.Identity,
                            bias=be2_sb[:, e, md : md + 1],
                        )
                        oslice = out_acc[:, md, tt * TT : (tt + 1) * TT]
                        wslice = w_bcast[:, e, tt * TT : (tt + 1) * TT]
                        if e == 0:
                            nc.vector.tensor_mul(oslice, y_sb[:], wslice)
                        else:
                            y2 = ytpool.tile([P, TT], F32, tag="ysb2")
                            nc.vector.tensor_mul(y2[:], y_sb[:], wslice)
                            nc.vector.tensor_add(oslice, oslice, y2[:])

            for md in range(KD):
                nc.sync.dma_start(
                    out=outT.ap()[md * P : (md + 1) * P, :], in_=out_acc[:, md, :]
                )

    return nc


def make_in_maps(x, rW1, rb1, rW2, rb2, We1, be1, We2, be2):
    x = np.ascontiguousarray(np.asarray(x, dtype=np.float32).reshape(B * L, D))
    shared = {
        "rW1": np.ascontiguousarray(np.asarray(rW1, np.float32)),
        "rb1": np.ascontiguousarray(np.asarray(rb1, np.float32)),
        "rW2": np.ascontiguousarray(np.asarray(rW2, np.float32)),
        "rb2": np.ascontiguousarray(np.asarray(rb2, np.float32)),
        "We1": np.ascontiguousarray(np.asarray(We1, np.float32)),
        "be1": np.ascontiguousarray(np.asarray(be1, np.float32)),
        "We2": np.ascontiguousarray(np.asarray(We2, np.float32)),
        "be2": np.ascontiguousarray(np.asarray(be2, np.float32)),
    }
    in_maps = []
    for c in range(NCORES):
        xT = np.ascontiguousarray(x[c * T : (c + 1) * T, :].T)
        in_maps.append({"xT": xT, **shared})
    return in_maps


def assemble_out(results):
    outs = [np.asarray(r["outT"]).T for r in results]
    return np.ascontiguousarray(np.concatenate(outs, axis=0).reshape(B, L, D)).astype(
        np.float32
    )


def kernel(x, rW1, rb1, rW2, rb2, We1, be1, We2, be2):
    from concourse.bass_utils import run_bass_kernel_spmd

    nc = build_nc()
    in_maps = make_in_maps(x, rW1, rb1, rW2, rb2, We1, be1, We2, be2)
    res = run_bass_kernel_spmd(nc, in_maps, core_ids=list(range(NCORES)))
    return assemble_out(res.results)


# revision 9
# speedup vs baseline: 1.1732x; 1.1732x over previous
"""CoLT5 MoE layer (router + top-2-of-4 experts) on 8 TRN2 NeuronCores.

Sharding: data-parallel over B*L = 8192 tokens -> 1024 tokens per core.
Each core computes the router and all 4 expert MLPs densely for its token
shard; combine weights (softmax probs masked to top-2) zero out non-selected
experts, which is numerically identical to the reference's masked-input
formulation (anything times a 0.0 gate is 0.0).

Device layout is feature-major ([feature=partitions, tokens=free]) so both
weight matrices are used in natural [in, out] layout as the stationary matmul
operand, and no activation transposes are needed on device. The host
transposes each x shard once on the way in and the output once on the way
out (pure layout prep; all FLOPs run on device).

Precision: router + expert layer 1 run in float32r (full-rate fp32 matmul
mode), expert layer 2 in bf16 (weights cast by the load DMA, hidden
activations cast by the GELU's output dtype). Routing decisions (top-2
selection) are made from float32r-precision logits.
"""

import sys

for _p in ("/opt/trn_rl_repo",):
    if _p not in sys.path:
        sys.path.insert(0, _p)

import numpy as np

import concourse.bass as bass
import concourse.mybir as mybir
import concourse.tile as tile
from concourse.masks import make_identity
from concourse.tile import TileContext
from concourse.vector_clock import ScopedClock

F32 = mybir.dt.float32
F32R = mybir.dt.float32r
BF16 = mybir.dt.bfloat16

B, L, D, E = 4, 2048, 1024, 4
DH = 2 * D          # router hidden = 2048
H = 4 * D           # expert hidden = 4096
NCORES = 8
T = (B * L) // NCORES   # tokens per core = 1024
TT = 512                # token tile (fp32 moving-operand max)
NTT = T // TT           # 2
P = 128
KD = D // P             # 8   D tiles
MH_R = DH // P          # 16  router-hidden tiles
MH = H // P             # 32  expert-hidden tiles
NCH = T // P            # 8   token chunks (for routing math)


def _patched_drain_and_barrier(self, tick_clock, wait_clock):
    # Workaround: this walrus build rejects >1 sync-wait attached to the
    # Tile kernel-tail Drain ("Too many sync wait commands",
    # CoreV3GenImpl setupSyncWait). Hang the waits on nop carriers, one
    # wait each, then drain.
    nop_inst = self.nc.sync.nop(nofuse=True)
    wait_clock.add_sem_waits(nop_inst.ins, ScopedClock({None: tick_clock.global_clock}))
    si = nop_inst.ins.sync_info
    waits = list(si.on_wait) if si else []
    if len(waits) > 1:
        si.on_wait = waits[:1]
        for w in waits[1:]:
            extra = self.nc.sync.nop(nofuse=True)
            extra.ins.sync_info = mybir.SyncInfo(on_wait=[w], on_update=[])
    self.nc.sync.drain()
    self.nc.all_engine_barrier()
    popped = self.nc._tile_sem_poison_stack.pop()
    assert popped is self._sem_poison
    self.nc.clear_and_free_semaphores(list(self.sems.allocated().values()))
    self.nc.all_engine_barrier()


tile.TileContext._drain_and_barrier = _patched_drain_and_barrier

_SPLIT_ENGINES = {"PE", "DVE", "Activation", "Pool", "SP"}


def _split_multi_waits(raw):
    # This walrus build accepts at most ONE sync-wait per instruction
    # ("Too many sync wait commands"). Move excess waits onto same-engine
    # NoOp carriers inserted immediately before the owning instruction.
    import json as _json

    d = _json.loads(raw)
    ctr = [0]

    def fix_block(b):
        ins_list = b.get("instructions")
        if ins_list:
            new_list = []
            for ins in ins_list:
                si = ins.get("sync_info")
                waits = (si or {}).get("on_wait") or []
                if len(waits) > 1 and ins.get("engine") in _SPLIT_ENGINES:
                    for w in waits[:-1]:
                        ctr[0] += 1
                        nop = {
                            "engine": ins["engine"],
                            "ins": [],
                            "outs": [],
                            "name": f"I-wsplit-{ctr[0]}",
                            "opcode": "NoOp",
                            "sync_info": {"on_update": [], "on_wait": [w]},
                        }
                        if "debug" in ins:
                            nop["debug"] = ins["debug"]
                        new_list.append(nop)
                    si["on_wait"] = [waits[-1]]
                new_list.append(ins)
            b["instructions"] = new_list
        for sub in b.get("blocks") or []:
            fix_block(sub)

    for f in d["functions"]:
        for b in f["blocks"]:
            fix_block(b)
    return _json.dumps(d).encode()


_orig_to_json_bytes = bass.Bass.to_json_bytes


def _patched_to_json_bytes(self):
    return _split_multi_waits(_orig_to_json_bytes(self))


bass.Bass.to_json_bytes = _patched_to_json_bytes


def build_nc():
    nc = bass.Bass("TRN2", target_bir_lowering=False, debug=False)

    xT = nc.dram_tensor("xT", [D, T], F32, kind="ExternalInput")
    rW1 = nc.dram_tensor("rW1", [D, DH], F32, kind="ExternalInput")
    rb1 = nc.dram_tensor("rb1", [DH], F32, kind="ExternalInput")
    rW2 = nc.dram_tensor("rW2", [DH, E], F32, kind="ExternalInput")
    rb2 = nc.dram_tensor("rb2", [E], F32, kind="ExternalInput")
    We1 = nc.dram_tensor("We1", [E, D, H], F32, kind="ExternalInput")
    be1 = nc.dram_tensor("be1", [E, H], F32, kind="ExternalInput")
    We2 = nc.dram_tensor("We2", [E, H, D], F32, kind="ExternalInput")
    be2 = nc.dram_tensor("be2", [E, D], F32, kind="ExternalInput")
    outT = nc.dram_tensor("outT", [D, T], F32, kind="ExternalOutput")

    AF = mybir.ActivationFunctionType
    ALU = mybir.AluOpType
    AX = mybir.AxisListType

    with TileContext(nc) as tc:
        from contextlib import ExitStack

        ctx = ExitStack()
        with ctx:
            # ---- long-lived pools -------------------------------------
            const = ctx.enter_context(tc.tile_pool(name="const", bufs=1))
            persist = ctx.enter_context(tc.tile_pool(name="persist", bufs=1))
            w1pool = ctx.enter_context(tc.tile_pool(name="w1pool", bufs=4))
            w2pool = ctx.enter_context(tc.tile_pool(name="w2pool", bufs=3))
            rhpool = ctx.enter_context(tc.tile_pool(name="rhpool", bufs=2))
            ytpool = ctx.enter_context(tc.tile_pool(name="ytpool", bufs=2))


            ident = const.tile([P, P], F32, tag="ident")
            make_identity(nc, ident)

            # biases, partition-major
            rb1_sb = const.tile([P, MH_R], F32, tag="rb1")
            nc.sync.dma_start(out=rb1_sb[:], in_=rb1.ap().rearrange("(a p) -> p a", p=P))
            rb2_sb = const.tile([E, 1], F32, tag="rb2")
            nc.sync.dma_start(out=rb2_sb[:], in_=rb2.ap().rearrange("(p a) -> p a", a=1))
            be1_sb = const.tile([P, E, MH], F32, tag="be1")
            nc.sync.dma_start(out=be1_sb[:], in_=be1.ap().rearrange("e (a p) -> p e a", p=P))
            be2_sb = const.tile([P, E, KD], F32, tag="be2")
            nc.sync.dma_start(out=be2_sb[:], in_=be2.ap().rearrange("e (a p) -> p e a", p=P))

            # resident activations
            xT_sb = persist.tile([P, KD, T], F32R, tag="xT")
            for kd in range(KD):
                nc.gpsimd.dma_start(
                    out=xT_sb[:, kd, :],
                    in_=xT.ap()[kd * P : (kd + 1) * P, :],
                )
            h_bf = persist.tile([P, MH, T], BF16, tag="h")
            out_acc = persist.tile([P, KD, T], F32, tag="outacc")
            w_bcast = persist.tile([P, E, T], F32, tag="wbc")

            rW2_sb = const.tile([P, MH_R, E], F32R, tag="rW2")
            nc.gpsimd.dma_start(out=rW2_sb[:], in_=rW2.ap().rearrange("(a p) e -> p a e", p=P))

            # ---- router ----------------------------------------------
            with (
                tc.tile_pool(name="plr", bufs=2, space="PSUM") as plr,
                tc.tile_pool(name="pl1r", bufs=3, space="PSUM") as pl1,
            ):
                logits_ps = [
                    plr.tile([E, TT], F32, tag="logits", name=f"logits{tt}")
                    for tt in range(NTT)
                ]
                for mh in range(MH_R):
                    w1blk = w1pool.tile([P, KD, P], F32R, tag="w1blk")
                    nc.gpsimd.dma_start(
                        out=w1blk[:],
                        in_=rW1.ap()[:, mh * P : (mh + 1) * P].rearrange(
                            "(kd p) h -> p kd h", p=P
                        ),
                    )
                    w1r = w1blk[:]
                    for tt in range(NTT):
                        ps1 = pl1.tile([P, TT], F32, tag="ps1")
                        for kd in range(KD):
                            nc.tensor.matmul(
                                ps1[:],
                                w1r[:, kd, :],
                                xT_sb[:, kd, tt * TT : (tt + 1) * TT],
                                start=(kd == 0),
                                stop=(kd == KD - 1),
                            )
                        rh_t = rhpool.tile([P, TT], F32R, tag="rh")
                        nc.scalar.activation(
                            rh_t[:], ps1[:], AF.Gelu, bias=rb1_sb[:, mh : mh + 1]
                        )
                        nc.tensor.matmul(
                            logits_ps[tt][:],
                            rW2_sb[:, mh, :],
                            rh_t[:],
                            start=(mh == 0),
                            stop=(mh == MH_R - 1),
                            skip_group_check=True,
                        )
                logits_sb = persist.tile([E, T], F32, tag="logits_sb")
                for tt in range(NTT):
                    nc.scalar.activation(
                        logits_sb[:, tt * TT : (tt + 1) * TT],
                        logits_ps[tt][:],
                        AF.Identity,
                        bias=rb2_sb[:],
                    )

            # ---- routing math (token-major) ---------------------------
            with (
                tc.tile_pool(name="ptp", bufs=2, space="PSUM") as ptp,
                tc.tile_pool(name="route", bufs=1) as route,
            ):
                ltm = route.tile([P, NCH, E], F32, tag="ltm")
                for c in range(NCH):
                    tp = ptp.tile([P, P], F32, tag="tp")
                    nc.tensor.transpose(
                        tp[:, :E], logits_sb[:, c * P : (c + 1) * P], ident[:E, :E]
                    )
                    nc.scalar.copy(ltm[:, c, :], tp[:, :E])

                m0 = route.tile([P, NCH, 1], F32, tag="m0")
                nc.vector.reduce_max(m0[:], ltm[:], axis=AX.X)
                sh = route.tile([P, NCH, E], F32, tag="sh")
                nc.vector.tensor_sub(sh[:], ltm[:], m0[:].to_broadcast([P, NCH, E]))
                ex = route.tile([P, NCH, E], F32, tag="ex")
                nc.scalar.activation(ex[:], sh[:], AF.Exp)
                ssum = route.tile([P, NCH, 1], F32, tag="ssum")
                nc.vector.reduce_sum(ssum[:], ex[:], axis=AX.X)
                rec = route.tile([P, NCH, 1], F32, tag="rec")
                nc.vector.reciprocal(rec[:], ssum[:])
                probs = route.tile([P, NCH, E], F32, tag="probs")
                nc.vector.tensor_mul(probs[:], ex[:], rec[:].to_broadcast([P, NCH, E]))

                m1 = route.tile([P, NCH, 1], F32, tag="m1")
                nc.vector.reduce_max(m1[:], probs[:], axis=AX.X)
                selmax = route.tile([P, NCH, E], F32, tag="selmax")
                nc.vector.tensor_tensor(
                    out=selmax[:], in0=probs[:], in1=m1[:].to_broadcast([P, NCH, E]),
                    op=ALU.is_ge,
                )
                masked = route.tile([P, NCH, E], F32, tag="masked")
                nc.vector.tensor_scalar_mul(selmax[:], selmax[:], 2.0)
                nc.vector.tensor_sub(masked[:], probs[:], selmax[:])
                m2 = route.tile([P, NCH, 1], F32, tag="m2")
                nc.vector.reduce_max(m2[:], masked[:], axis=AX.X)
                sel = route.tile([P, NCH, E], F32, tag="sel")
                nc.vector.tensor_tensor(
                    out=sel[:], in0=probs[:], in1=m2[:].to_broadcast([P, NCH, E]),
                    op=ALU.is_ge,
                )
                combine = route.tile([P, NCH, E], F32, tag="combine")
                nc.vector.tensor_mul(combine[:], probs[:], sel[:])

                # transpose-and-broadcast combine back to feature-major:
                # out[m, t] = combine[t, e] via a matmul whose stationary
                # operand is one combine column broadcast along its free dim
                # (step 0), against the identity.
                for e in range(E):
                    for c in range(NCH):
                        bc = ptp.tile([P, P], F32, tag="tp")
                        nc.tensor.matmul(
                            bc[:],
                            combine[:, c, e : e + 1].to_broadcast([P, P]),
                            ident[:],
                            start=True,
                            stop=True,
                        )
                        nc.scalar.copy(w_bcast[:, e, c * P : (c + 1) * P], bc[:])

            # ---- experts ---------------------------------------------
            pl1 = ctx.enter_context(tc.tile_pool(name="pl1", bufs=4, space="PSUM"))
            pl2 = ctx.enter_context(tc.tile_pool(name="pl2", bufs=4, space="PSUM"))
            for e in range(E):
                for mh in range(MH):
                    w1blk = w1pool.tile([P, KD, P], F32R, tag="w1blk")
                    nc.gpsimd.dma_start(
                        out=w1blk[:],
                        in_=We1.ap()[e, :, mh * P : (mh + 1) * P].rearrange(
                            "(kd p) h -> p kd h", p=P
                        ),
                    )
                    w1r = w1blk[:]
                    for tt in range(NTT):
                        ps1 = pl1.tile([P, TT], F32, tag="ps1")
                        for kd in range(KD):
                            nc.tensor.matmul(
                                ps1[:],
                                w1r[:, kd, :],
                                xT_sb[:, kd, tt * TT : (tt + 1) * TT],
                                start=(kd == 0),
                                stop=(kd == KD - 1),
                            )
                        nc.scalar.activation(
                            h_bf[:, mh, tt * TT : (tt + 1) * TT],
                            ps1[:],
                            AF.Gelu,
                            bias=be1_sb[:, e, mh : mh + 1],
                        )
                for md in range(KD):
                    w2blk = w2pool.tile([P, MH, P], BF16, tag="w2blk")
                    nc.gpsimd.dma_start(
                        out=w2blk[:],
                        in_=We2.ap()[e, :, md * P : (md + 1) * P].rearrange(
                            "(mh p) d -> p mh d", p=P
                        ),
                    )
                    for tt in range(NTT):
                        ps2 = pl2.tile([P, TT], F32, tag="ps2")
                        for mh in range(MH):
                            nc.tensor.matmul(
                                ps2[:],
                                w2blk[:, mh, :],
                                h_bf[:, mh, tt * TT : (tt + 1) * TT],
                                start=(mh == 0),
                                stop=(mh == MH - 1),
                            )
                        y_sb = ytpool.tile([P, TT], F32, tag="ysb")
                        nc.scalar.activation(
                            y_sb[:], ps2[:], AF.Identity,
                            bias=be2_sb[:, e, md : md + 1],
                        )
                        oslice = out_acc[:, md, tt * TT : (tt + 1) * TT]
                        wslice = w_bcast[:, e, tt * TT : (tt + 1) * TT]
                        if e == 0:
                            nc.vector.tensor_mul(oslice, y_sb[:], wslice)
                        else:
                            y2 = ytpool.tile([P, TT], F32, tag="ysb2")
                            nc.vector.tensor_mul(y2[:], y_sb[:], wslice)
                            nc.vector.tensor_add(oslice, oslice, y2[:])

            for md in range(KD):
                nc.sync.dma_start(
                    out=outT.ap()[md * P : (md + 1) * P, :], in_=out_acc[:, md, :]
                )

    return nc


def make_in_maps(x, rW1, rb1, rW2, rb2, We1, be1, We2, be2):
    x = np.ascontiguousarray(np.asarray(x, dtype=np.float32).reshape(B * L, D))
    shared = {
        "rW1": np.ascontiguousarray(np.asarray(rW1, np.float32)),
        "rb1": np.ascontiguousarray(np.asarray(rb1, np.float32)),
        "rW2": np.ascontiguousarray(np.asarray(rW2, np.float32)),
        "rb2": np.ascontiguousarray(np.asarray(rb2, np.float32)),
        "We1": np.ascontiguousarray(np.asarray(We1, np.float32)),
        "be1": np.ascontiguousarray(np.asarray(be1, np.float32)),
        "We2": np.ascontiguousarray(np.asarray(We2, np.float32)),
        "be2": np.ascontiguousarray(np.asarray(be2, np.float32)),
    }
    in_maps = []
    for c in range(NCORES):
        xT = np.ascontiguousarray(x[c * T : (c + 1) * T, :].T)
        in_maps.append({"xT": xT, **shared})
    return in_maps


def assemble_out(results):
    outs = [np.asarray(r["outT"]).T for r in results]
    return np.ascontiguousarray(np.concatenate(outs, axis=0).reshape(B, L, D)).astype(
        np.float32
    )


def kernel(x, rW1, rb1, rW2, rb2, We1, be1, We2, be2):
    from concourse.bass_utils import run_bass_kernel_spmd

    nc = build_nc()
    in_maps = make_in_maps(x, rW1, rb1, rW2, rb2, We1, be1, We2, be2)
    res = run_bass_kernel_spmd(nc, in_maps, core_ids=list(range(NCORES)))
    return assemble_out(res.results)
